# revision 14
# baseline (speedup 1.0000x reference)
"""GPTQ-Marlin sparse MoE layer for 8 Trainium2 NeuronCores.

Strategy (expert-parallel + d_ff-slice rebalancing, host-side dispatch):
  - Router (softmax + top-2 + renormalize) replicates the reference with the
    same jax ops so expert selection matches bit-for-bit.
  - Phase 1: core e owns expert e and runs the FFN for the first C1 tokens of
    that expert (C1 chosen to minimize per-core work; NOT padded to 128).
  - Phase 2: the leftover tokens (experts with more than C1 tokens) are cut
    into 128-token blocks x 8 d_ff slices of 512 -> identical "units"
    distributed round-robin so every core gets the same number. A unit
    computes gate/up/act for its f-slice and a partial down-projection; the
    host sums the slice partials during the scatter-add combine (free).
  - GPTQ int4 codes are dequantized to bf16 on the host; weights stream from
    HBM in fine-grained tiles so the first matmul starts ~5us into the run.
  - All matmuls run transposed (tokens on the free dim): bf16 with fp32 PSUM.

Per-core tensor time ~ (C1 + 16 * units_per_core) token-equivalents vs the
naive max-expert padding; for typical routing this is ~1058 vs 1152.
"""

import numpy as np
import ml_dtypes

E, T, D, F, TOPK, GROUP = 8, 4096, 1024, 8192 // 2, 2, 128
P = 128
KO1 = D // P            # 8 k-tiles for mm1
FH = F // P             # 32 act tiles (k-tiles for mm2)
DO = D // P             # 8 output d-tiles
NJ = (2 * F) // P // 2  # 32 gate/up column-tile pairs for mm1
TC = 512                # token chunk (one PSUM bank of fp32)
FSL = 512               # phase-2 d_ff slice width
NSL = F // FSL          # 8 slices per expert
FH2 = FSL // P          # 4 act tiles per slice
NJ2 = FSL // P          # 4 gate/up pairs per slice

F8 = 192                # fp8 tokens per expert (smallest top-k coefs)
KO2 = KO1 // 2          # 4 double-row k-tiles (256 rows each) for fp8 mm1
FH8 = FH // 2           # 16 double-row k-tiles for fp8 mm2

LAST_RESULTS = None     # test harness introspection

_BUILD_CACHE = {}


def _route(gating_output):
    """softmax + top-k + renormalize, replicated exactly like the reference."""
    try:
        import jax
        import jax.numpy as jnp

        scores = jax.nn.softmax(jnp.asarray(gating_output, jnp.float32), axis=-1)
        topk_w, topk_ids = jax.lax.top_k(scores, TOPK)
        topk_w = topk_w / jnp.sum(topk_w, axis=-1, keepdims=True)
        return np.asarray(topk_w, np.float32), np.asarray(topk_ids)
    except Exception:
        g = np.asarray(gating_output, np.float32)
        ex = np.exp(g - g.max(axis=-1, keepdims=True))
        s = (ex / ex.sum(axis=-1, keepdims=True)).astype(np.float32)
        ids = np.argsort(-s, axis=-1, kind="stable")[:, :TOPK]
        w = np.take_along_axis(s, ids, axis=-1)
        w = (w / w.sum(axis=-1, keepdims=True)).astype(np.float32)
        return w, ids


def _dequant_bf16(q, s):
    """q: [K, N] int codes, s: [K//GROUP, N] scales -> bf16 [K, N]."""
    w = (np.asarray(q, np.float32) - 8.0) * np.repeat(
        np.asarray(s, np.float32), GROUP, axis=0
    )
    return w.astype(ml_dtypes.bfloat16)


def _chunks(C):
    out, t0 = [], 0
    while t0 < C:
        w = min(TC, C - t0)
        out.append((t0, w))
        t0 += w
    return out


def _build(C1, UPC, NF8):
    """Per-core program: phase-1 FFN for C1 tokens of one expert (NF8 of them
    on a full-fp8 double-row path), then UPC phase-2 units (128 tokens x 512
    d_ff slice each, possibly zero-padded)."""
    import concourse.mybir as mybir
    import concourse.tile as tile
    from concourse import bacc

    nc = bacc.Bacc("TRN2", name="moe_expert_ffn")
    bf16 = mybir.dt.bfloat16
    f32 = mybir.dt.float32
    fp8 = mybir.dt.float8e4
    DR = mybir.MatmulPerfMode.DoubleRow

    NB1 = C1 - NF8          # bf16 tokens in phase 1
    tcs = _chunks(NB1)

    xT = nc.dram_tensor("xT", [P, KO1, NB1], bf16, kind="ExternalInput")
    # [p, jj, gate/up, k, col]
    w1 = nc.dram_tensor("w1", [P, NJ, 2, KO1, P], bf16, kind="ExternalInput")
    w2 = nc.dram_tensor("w2", [P, DO, FH, P], bf16, kind="ExternalInput")
    yT = nc.dram_tensor("yT", [P, DO, NB1], f32, kind="ExternalOutput")
    if NF8:
        x8T = nc.dram_tensor("x8T", [P, KO2, 2, NF8], fp8, kind="ExternalInput")
        # [p, jj, gate/up, kk, pair, col]
        w1f8 = nc.dram_tensor("w1f8", [P, NJ, 2, KO2, 2, P], fp8,
                              kind="ExternalInput")
        # [p, d, kk, pair, col]
        w2f8 = nc.dram_tensor("w2f8", [P, DO, FH8, 2, P], fp8,
                              kind="ExternalInput")
        y8T = nc.dram_tensor("y8T", [P, DO, NF8], f32, kind="ExternalOutput")
    if UPC:
        x2T = nc.dram_tensor("x2T", [P, KO1, UPC * P], bf16, kind="ExternalInput")
        # [p, unit, jj2, gate/up, k, col]
        w1s = nc.dram_tensor("w1s", [P, UPC, NJ2, 2, KO1, P], bf16,
                             kind="ExternalInput")
        # [p, unit, k2, d, col]
        w2s = nc.dram_tensor("w2s", [P, UPC, FH2, DO, P], bf16,
                             kind="ExternalInput")
        y2T = nc.dram_tensor("y2T", [P, UPC, DO, P], f32, kind="ExternalOutput")

    with tile.TileContext(nc) as tc:
        with (
            tc.tile_pool(name="xpool", bufs=1) as xpool,
            tc.tile_pool(name="w1pool", bufs=4) as w1pool,
            tc.tile_pool(name="w2pool", bufs=2) as w2pool,
            tc.tile_pool(name="actpool", bufs=1) as actpool,
            tc.tile_pool(name="sgpool", bufs=2) as sgpool,
            tc.tile_pool(name="ypool", bufs=3) as ypool,
            tc.tile_pool(name="pspool", bufs=5, space="PSUM") as pspool,
            tc.tile_pool(name="psypool", bufs=2, space="PSUM") as psypool,
        ):
            # Warm the PE clock (pstate ramp) with throwaway matmuls while the
            # first DMAs land; results are never read.
            junk = xpool.tile([P, TC], bf16, tag="junk")
            nc.vector.memset(junk[:], 0)
            psw = pspool.tile([P, TC], f32, tag="ps")
            for _ in range(10):
                nc.tensor.matmul(psw[:], junk[:, :P], junk[:], start=True,
                                 stop=True)

            # x streams in per token-chunk (own DMA queue via gpsimd) so the
            # first matmul only waits for chunk 0 plus the first weight tile.
            xcs = []
            for ci, (t0, tw) in enumerate(tcs):
                xc = xpool.tile([P, KO1, tw], bf16, tag=f"x{ci}")
                nc.gpsimd.dma_start(xc[:], xT[:, :, t0:t0 + tw])
                xcs.append(xc)
            if NF8:
                x8 = xpool.tile([P, KO2, 2, NF8], fp8, tag="x8")
                nc.gpsimd.dma_start(x8[:], x8T[:])

            act = actpool.tile([P, FH, NB1], bf16)
            if NF8:
                act8 = actpool.tile([P, FH8, 2, NF8], fp8, tag="act8")

            # ---- phase 1 mm1: h^T = W1^T x; act = silu(gate) * up ----
            for jj in range(NJ):
                wg = w1pool.tile([P, KO1, P], bf16, tag="w1t")
                nc.sync.dma_start(wg[:], w1[:, jj, 0])
                wu = w1pool.tile([P, KO1, P], bf16, tag="w1t")
                nc.sync.dma_start(wu[:], w1[:, jj, 1])
                if NF8:
                    w18 = w1pool.tile([P, 2, KO2, 2, P], fp8, tag="w18",
                                      bufs=4)
                    nc.gpsimd.dma_start(w18[:], w1f8[:, jj])
                for ci, (t0, tw) in enumerate(tcs):
                    psg = pspool.tile([P, TC], f32, tag="ps")
                    psu = pspool.tile([P, TC], f32, tag="ps")
                    for k in range(KO1):
                        nc.tensor.matmul(
                            psg[:, :tw], wg[:, k], xcs[ci][:, k],
                            start=(k == 0), stop=(k == KO1 - 1),
                        )
                    for k in range(KO1):
                        nc.tensor.matmul(
                            psu[:, :tw], wu[:, k], xcs[ci][:, k],
                            start=(k == 0), stop=(k == KO1 - 1),
                        )
                    sg = sgpool.tile([P, TC], f32, tag="sg")
                    nc.scalar.activation(
                        sg[:, :tw], psg[:, :tw],
                        mybir.ActivationFunctionType.Silu,
                    )
                    nc.vector.tensor_tensor(
                        act[:, jj, t0:t0 + tw], sg[:, :tw], psu[:, :tw],
                        mybir.AluOpType.mult,
                    )
                if NF8:
                    psg = pspool.tile([P, TC], f32, tag="ps")
                    psu = pspool.tile([P, TC], f32, tag="ps")
                    for kk in range(KO2):
                        nc.tensor.matmul(
                            psg[:, :NF8], w18[:, 0, kk], x8[:, kk],
                            start=(kk == 0), stop=(kk == KO2 - 1),
                            perf_mode=DR,
                        )
                    for kk in range(KO2):
                        nc.tensor.matmul(
                            psu[:, :NF8], w18[:, 1, kk], x8[:, kk],
                            start=(kk == 0), stop=(kk == KO2 - 1),
                            perf_mode=DR,
                        )
                    sg = sgpool.tile([P, TC], f32, tag="sg")
                    nc.scalar.activation(
                        sg[:, :NF8], psg[:, :NF8],
                        mybir.ActivationFunctionType.Silu,
                    )
                    nc.vector.tensor_tensor(
                        act8[:, jj // 2, jj % 2], sg[:, :NF8], psu[:, :NF8],
                        mybir.AluOpType.mult,
                    )

            # ---- phase 1 mm2: y^T = W2^T act ----
            for d in range(DO):
                ws = w2pool.tile([P, FH, P], bf16, tag="w2t")
                nc.sync.dma_start(ws[:], w2[:, d])
                if NF8:
                    w28 = w2pool.tile([P, FH8, 2, P], fp8, tag="w28", bufs=2)
                    nc.gpsimd.dma_start(w28[:], w2f8[:, d])
                for ci, (t0, tw) in enumerate(tcs):
                    psy = psypool.tile([P, TC], f32, tag="psy")
                    for k2 in range(FH):
                        nc.tensor.matmul(
                            psy[:, :tw], ws[:, k2], act[:, k2, t0:t0 + tw],
                            start=(k2 == 0), stop=(k2 == FH - 1),
                        )
                    yo = ypool.tile([P, TC], f32, tag="yo")
                    nc.scalar.copy(yo[:, :tw], psy[:, :tw])
                    nc.sync.dma_start(yT[:, d, t0:t0 + tw], yo[:, :tw])
                if NF8:
                    psy = psypool.tile([P, TC], f32, tag="psy")
                    for kk in range(FH8):
                        nc.tensor.matmul(
                            psy[:, :NF8], w28[:, kk], act8[:, kk],
                            start=(kk == 0), stop=(kk == FH8 - 1),
                            perf_mode=DR,
                        )
                    yo = ypool.tile([P, TC], f32, tag="yo")
                    nc.scalar.copy(yo[:, :NF8], psy[:, :NF8])
                    nc.sync.dma_start(y8T[:, d], yo[:, :NF8])

            # ---- phase 2: leftover-token units (128 tok x 512 d_ff) ----
            if UPC:
                for u in range(UPC):
                    if u < len(w1u_t):
                        w1u, w2u = w1u_t[u], w2u_t[u]
                    else:
                        w1u = w1pool.tile([P, NJ2, 2, KO1, P], bf16,
                                          tag="w1sr", bufs=2)
                        nc.gpsimd.dma_start(w1u[:], w1s[:, u])
                        w2u = w2pool.tile([P, FH2, DO, P], bf16,
                                          tag="w2sr", bufs=2)
                        nc.gpsimd.dma_start(w2u[:], w2s[:, u])
                    act2 = actpool.tile([P, FH2, P], bf16, tag="act2", bufs=2)
                    xu = x2[:, :, u * P:(u + 1) * P]
                    for jj in range(NJ2):
                        psg = pspool.tile([P, TC], f32, tag="ps")
                        psu = pspool.tile([P, TC], f32, tag="ps")
                        for k in range(KO1):
                            nc.tensor.matmul(
                                psg[:, :P], w1u[:, jj, 0, k], xu[:, k],
                                start=(k == 0), stop=(k == KO1 - 1),
                            )
                        for k in range(KO1):
                            nc.tensor.matmul(
                                psu[:, :P], w1u[:, jj, 1, k], xu[:, k],
                                start=(k == 0), stop=(k == KO1 - 1),
                            )
                        sg = sgpool.tile([P, TC], f32, tag="sg")
                        nc.scalar.activation(
                            sg[:, :P], psg[:, :P],
                            mybir.ActivationFunctionType.Silu,
                        )
                        nc.vector.tensor_tensor(
                            act2[:, jj], sg[:, :P], psu[:, :P],
                            mybir.AluOpType.mult,
                        )
                    for d in range(DO):
                        psy = psypool.tile([P, TC], f32, tag="psy")
                        for k2 in range(FH2):
                            nc.tensor.matmul(
                                psy[:, :P], w2u[:, k2, d], act2[:, k2],
                                start=(k2 == 0), stop=(k2 == FH2 - 1),
                            )
                        yo = ypool.tile([P, TC], f32, tag="yo")
                        nc.scalar.copy(yo[:, :P], psy[:, :P])
                        nc.sync.dma_start(y2T[:, u, d], yo[:, :P])
    return nc


def _plan(counts):
    """Pick C1 and the phase-2 unit list minimizing per-core token-equivs."""
    cmax = max(counts)
    best = None
    for C1 in range(min(P, cmax), cmax + 1):
        blocks = sum((max(c - C1, 0) + P - 1) // P for c in counts)
        units = blocks * NSL
        upc = (units + E - 1) // E
        cost = C1 + (P * FSL // F) * upc  # C1 + 16 * units-per-core
        if best is None or cost < best[0] or (cost == best[0] and C1 > best[1]):
            best = (cost, C1, upc)
    _, C1, upc = best
    return C1, upc


def _pack_w1_phase1(w1d):
    # w1d [D, 2F] -> [P, NJ, 2, KO1, P]
    g = w1d[:, :F].reshape(KO1, P, NJ, P)       # [k, p, jj, c]
    u = w1d[:, F:].reshape(KO1, P, NJ, P)
    out = np.stack([g.transpose(1, 2, 0, 3), u.transpose(1, 2, 0, 3)], axis=2)
    return np.ascontiguousarray(out)            # [p, jj, gu, k, c]


def _pack_w2_phase1(w2d):
    # w2d [F, D] -> [P, DO, FH, P]
    return np.ascontiguousarray(w2d.reshape(FH, P, DO, P).transpose(1, 2, 0, 3))


def _pack_xT(xe, C):
    # xe [C, D] -> [P, KO1, C]
    return np.ascontiguousarray(xe.T.reshape(KO1, P, C).transpose(1, 0, 2))


def kernel(x, gating_output, w1_q, w2_q, w1_scale, w2_scale):
    global LAST_RESULTS
    from concourse.bass_utils import run_bass_kernel_spmd

    x = np.asarray(x, np.float32)
    w1_q = np.asarray(w1_q)
    w2_q = np.asarray(w2_q)
    w1_scale = np.asarray(w1_scale, np.float32)
    w2_scale = np.asarray(w2_scale, np.float32)

    topk_w, topk_ids = _route(gating_output)

    token_lists, coefs = [], []
    for e in range(E):
        mask = topk_ids == e
        tok = np.nonzero(mask.any(axis=1))[0]
        cf = np.where(mask, topk_w, 0.0).sum(axis=1)[tok].astype(np.float32)
        token_lists.append(tok)
        coefs.append(cf)

    counts = [len(t) for t in token_lists]
    C1, UPC = _plan(counts)

    key = (C1, UPC)
    if key not in _BUILD_CACHE:
        nc = _build(C1, UPC)
        nc.finalize()
        _BUILD_CACHE[key] = nc
    nc = _BUILD_CACHE[key]

    # host-side dequant (once per expert)
    w1ds = [_dequant_bf16(w1_q[e], w1_scale[e]) for e in range(E)]  # [D, 2F]
    w2ds = [_dequant_bf16(w2_q[e], w2_scale[e]) for e in range(E)]  # [F, D]

    # phase-2 unit list: (expert, block_start_in_tok_list, slice)
    units = []
    for e in range(E):
        left = counts[e] - C1
        b0 = C1
        while left > 0:
            for s in range(NSL):
                units.append((e, b0, s))
            b0 += P
            left -= P
    per_core_units = [[] for _ in range(E)]
    for i, unit in enumerate(units):
        per_core_units[i % E].append(unit)

    in_maps = []
    for c in range(E):
        tok = token_lists[c][:C1]
        xe = np.zeros((C1, D), np.float32)
        xe[: len(tok)] = x[tok]
        im = {
            "xT": _pack_xT(xe.astype(ml_dtypes.bfloat16), C1),
            "w1": _pack_w1_phase1(w1ds[c]),
            "w2": _pack_w2_phase1(w2ds[c]),
        }
        if UPC:
            x2 = np.zeros((UPC * P, D), np.float32)
            w1s = np.zeros((P, UPC, NJ2, 2, KO1, P), ml_dtypes.bfloat16)
            w2s = np.zeros((P, UPC, FH2, DO, P), ml_dtypes.bfloat16)
            for j, (e, b0, s) in enumerate(per_core_units[c]):
                btok = token_lists[e][b0:b0 + P]
                x2[j * P: j * P + len(btok)] = x[btok]
                w1d = w1ds[e]
                gs = w1d[:, s * FSL:(s + 1) * FSL].reshape(KO1, P, NJ2, P)
                us = w1d[:, F + s * FSL: F + (s + 1) * FSL].reshape(KO1, P, NJ2, P)
                w1s[:, j] = np.stack(
                    [gs.transpose(1, 2, 0, 3), us.transpose(1, 2, 0, 3)], axis=2
                )
                w2sl = w2ds[e][s * FSL:(s + 1) * FSL]  # [FSL, D]
                w2s[:, j] = w2sl.reshape(FH2, P, DO, P).transpose(1, 0, 2, 3)
            im["x2T"] = _pack_xT(x2.astype(ml_dtypes.bfloat16), UPC * P)
            im["w1s"] = w1s
            im["w2s"] = w2s
        in_maps.append(im)

    LAST_RESULTS = run_bass_kernel_spmd(nc, in_maps, core_ids=list(range(E)))

    out = np.zeros((T, D), np.float32)
    for c in range(E):
        yTe = LAST_RESULTS.results[c]["yT"]          # [P, DO, C1] f32
        y = yTe.transpose(1, 0, 2).reshape(D, C1).T  # [C1, D]
        tok = token_lists[c][:C1]
        out[tok] += coefs[c][: len(tok), None] * y[: len(tok)]
        if UPC:
            y2Te = LAST_RESULTS.results[c]["y2T"]    # [P, UPC, DO, P] f32
            for j, (e, b0, s) in enumerate(per_core_units[c]):
                btok = token_lists[e][b0:b0 + P]
                y2 = y2Te[:, j].transpose(1, 0, 2).reshape(D, P).T  # [P, D]
                out[btok] += coefs[e][b0:b0 + len(btok), None] * y2[: len(btok)]
    return out


# revision 21
# speedup vs baseline: 1.1122x; 1.1122x over previous
"""GPTQ-Marlin sparse MoE layer for 8 Trainium2 NeuronCores.

Strategy (expert-parallel + d_ff-slice rebalancing, host-side dispatch):
  - Router (softmax + top-2 + renormalize) replicates the reference with the
    same jax ops so expert selection matches bit-for-bit.
  - Phase 1: core e owns expert e and runs the FFN for the first C1 tokens of
    that expert (C1 chosen to minimize per-core work; NOT padded to 128).
  - Phase 2: the leftover tokens (experts with more than C1 tokens) are cut
    into 128-token blocks x 8 d_ff slices of 512 -> identical "units"
    distributed round-robin so every core gets the same number. A unit
    computes gate/up/act for its f-slice and a partial down-projection; the
    host sums the slice partials during the scatter-add combine (free).
  - GPTQ int4 codes are dequantized to bf16 on the host; weights stream from
    HBM in fine-grained tiles so the first matmul starts ~5us into the run.
  - All matmuls run transposed (tokens on the free dim): bf16 with fp32 PSUM.

Per-core tensor time ~ (C1 + 16 * units_per_core) token-equivalents vs the
naive max-expert padding; for typical routing this is ~1058 vs 1152.
"""

import numpy as np
import ml_dtypes

E, T, D, F, TOPK, GROUP = 8, 4096, 1024, 8192 // 2, 2, 128
P = 128
KO1 = D // P            # 8 k-tiles for mm1
FH = F // P             # 32 act tiles (k-tiles for mm2)
DO = D // P             # 8 output d-tiles
NJ = (2 * F) // P // 2  # 32 gate/up column-tile pairs for mm1
TC = 512                # token chunk (one PSUM bank of fp32)
FSL = 512               # phase-2 d_ff slice width
NSL = F // FSL          # 8 slices per expert
FH2 = FSL // P          # 4 act tiles per slice
NJ2 = FSL // P          # 4 gate/up pairs per slice

F8 = 192                # fp8 tokens per expert (smallest top-k coefs)
KO2 = KO1 // 2          # 4 double-row k-tiles (256 rows each) for fp8 mm1
FH8 = FH // 2           # 16 double-row k-tiles for fp8 mm2

LAST_RESULTS = None     # test harness introspection

_BUILD_CACHE = {}


def _route(gating_output):
    """softmax + top-k + renormalize, replicated exactly like the reference."""
    try:
        import jax
        import jax.numpy as jnp

        scores = jax.nn.softmax(jnp.asarray(gating_output, jnp.float32), axis=-1)
        topk_w, topk_ids = jax.lax.top_k(scores, TOPK)
        topk_w = topk_w / jnp.sum(topk_w, axis=-1, keepdims=True)
        return np.asarray(topk_w, np.float32), np.asarray(topk_ids)
    except Exception:
        g = np.asarray(gating_output, np.float32)
        ex = np.exp(g - g.max(axis=-1, keepdims=True))
        s = (ex / ex.sum(axis=-1, keepdims=True)).astype(np.float32)
        ids = np.argsort(-s, axis=-1, kind="stable")[:, :TOPK]
        w = np.take_along_axis(s, ids, axis=-1)
        w = (w / w.sum(axis=-1, keepdims=True)).astype(np.float32)
        return w, ids


def _dequant_bf16(q, s):
    """q: [K, N] int codes, s: [K//GROUP, N] scales -> bf16 [K, N]."""
    w = (np.asarray(q, np.float32) - 8.0) * np.repeat(
        np.asarray(s, np.float32), GROUP, axis=0
    )
    return w.astype(ml_dtypes.bfloat16)


def _chunks(C):
    out, t0 = [], 0
    while t0 < C:
        w = min(TC, C - t0)
        out.append((t0, w))
        t0 += w
    return out


def _build(C1, UPC, NF8):
    """Per-core program: phase-1 FFN for C1 tokens of one expert (NF8 of them
    on a full-fp8 double-row path), then UPC phase-2 units (128 tokens x 512
    d_ff slice each, possibly zero-padded)."""
    import concourse.mybir as mybir
    import concourse.tile as tile
    from concourse import bacc

    nc = bacc.Bacc("TRN2", name="moe_expert_ffn")
    bf16 = mybir.dt.bfloat16
    f32 = mybir.dt.float32
    fp8 = mybir.dt.float8e4
    DR = mybir.MatmulPerfMode.DoubleRow

    NB1 = C1 - NF8          # bf16 tokens in phase 1
    tcs = _chunks(NB1)

    xT = nc.dram_tensor("xT", [P, KO1, NB1], bf16, kind="ExternalInput")
    # [p, jj, gate/up, k, col]
    w1 = nc.dram_tensor("w1", [P, NJ, 2, KO1, P], bf16, kind="ExternalInput")
    w2 = nc.dram_tensor("w2", [P, DO, FH, P], bf16, kind="ExternalInput")
    yT = nc.dram_tensor("yT", [P, DO, NB1], f32, kind="ExternalOutput")
    if NF8:
        x8T = nc.dram_tensor("x8T", [P, KO2, 2, NF8], fp8, kind="ExternalInput")
        # [p, jj, gate/up, kk, pair, col]
        w1f8 = nc.dram_tensor("w1f8", [P, NJ, 2, KO2, 2, P], fp8,
                              kind="ExternalInput")
        # [p, d, kk, pair, col]
        w2f8 = nc.dram_tensor("w2f8", [P, DO, FH8, 2, P], fp8,
                              kind="ExternalInput")
        y8T = nc.dram_tensor("y8T", [P, DO, NF8], f32, kind="ExternalOutput")
    if UPC:
        x2T = nc.dram_tensor("x2T", [P, KO1, UPC * P], bf16, kind="ExternalInput")
        # [p, unit, jj2, gate/up, k, col]
        w1s = nc.dram_tensor("w1s", [P, UPC, NJ2, 2, KO1, P], bf16,
                             kind="ExternalInput")
        # [p, unit, k2, d, col]
        w2s = nc.dram_tensor("w2s", [P, UPC, FH2, DO, P], bf16,
                             kind="ExternalInput")
        y2T = nc.dram_tensor("y2T", [P, UPC, DO, P], f32, kind="ExternalOutput")

    with tile.TileContext(nc) as tc:
        with (
            tc.tile_pool(name="xpool", bufs=1) as xpool,
            tc.tile_pool(name="w1pool", bufs=4) as w1pool,
            tc.tile_pool(name="w2pool", bufs=2) as w2pool,
            tc.tile_pool(name="actpool", bufs=1) as actpool,
            tc.tile_pool(name="sgpool", bufs=2) as sgpool,
            tc.tile_pool(name="ypool", bufs=3) as ypool,
            tc.tile_pool(name="pspool", bufs=5, space="PSUM") as pspool,
            tc.tile_pool(name="psypool", bufs=2, space="PSUM") as psypool,
        ):
            # x streams in per token-chunk; chunk 0 first on the sync queue so
            # the first matmul waits only for it plus the jj=0 weight tiles
            # (xc1's dma is issued just after those, inside the jj loop).
            xcs = []
            for ci, (t0, tw) in enumerate(tcs):
                xc = xpool.tile([P, KO1, tw], bf16, tag=f"x{ci}")
                if ci == 0:
                    nc.sync.dma_start(xc[:], xT[:, :, t0:t0 + tw])
                xcs.append(xc)
            if NF8:
                x8 = xpool.tile([P, KO2, 2, NF8], fp8, tag="x8")
                nc.scalar.dma_start(x8[:], x8T[:])

            act = actpool.tile([P, FH, NB1], bf16)
            if NF8:
                act8 = actpool.tile([P, FH8, 2, NF8], fp8, tag="act8")

            # ---- phase 1 mm1: h^T = W1^T x; act = silu(gate) * up ----
            for jj in range(NJ):
                wg = w1pool.tile([P, KO1, P], bf16, tag="w1t")
                nc.sync.dma_start(wg[:], w1[:, jj, 0])
                wu = w1pool.tile([P, KO1, P], bf16, tag="w1t")
                nc.sync.dma_start(wu[:], w1[:, jj, 1])
                if NF8:
                    w18 = w1pool.tile([P, 2, KO2, 2, P], fp8, tag="w18",
                                      bufs=4)
                    nc.scalar.dma_start(w18[:], w1f8[:, jj])
                for ci, (t0, tw) in enumerate(tcs):
                    if jj == 0 and ci > 0:
                        nc.sync.dma_start(xcs[ci][:], xT[:, :, t0:t0 + tw])
                for ci, (t0, tw) in enumerate(tcs):
                    psg = pspool.tile([P, TC], f32, tag="ps")
                    psu = pspool.tile([P, TC], f32, tag="ps")
                    for k in range(KO1):
                        nc.tensor.matmul(
                            psg[:, :tw], wg[:, k], xcs[ci][:, k],
                            start=(k == 0), stop=(k == KO1 - 1),
                        )
                    for k in range(KO1):
                        nc.tensor.matmul(
                            psu[:, :tw], wu[:, k], xcs[ci][:, k],
                            start=(k == 0), stop=(k == KO1 - 1),
                        )
                    sg = sgpool.tile([P, TC], f32, tag="sg")
                    nc.scalar.activation(
                        sg[:, :tw], psg[:, :tw],
                        mybir.ActivationFunctionType.Silu,
                    )
                    nc.vector.tensor_tensor(
                        act[:, jj, t0:t0 + tw], sg[:, :tw], psu[:, :tw],
                        mybir.AluOpType.mult,
                    )
                if NF8:
                    psg = pspool.tile([P, TC], f32, tag="ps")
                    psu = pspool.tile([P, TC], f32, tag="ps")
                    for kk in range(KO2):
                        nc.tensor.matmul(
                            psg[:, :NF8], w18[:, 0, kk], x8[:, kk],
                            start=(kk == 0), stop=(kk == KO2 - 1),
                            perf_mode=DR,
                        )
                    for kk in range(KO2):
                        nc.tensor.matmul(
                            psu[:, :NF8], w18[:, 1, kk], x8[:, kk],
                            start=(kk == 0), stop=(kk == KO2 - 1),
                            perf_mode=DR,
                        )
                    sg = sgpool.tile([P, TC], f32, tag="sg")
                    nc.scalar.activation(
                        sg[:, :NF8], psg[:, :NF8],
                        mybir.ActivationFunctionType.Silu,
                    )
                    nc.vector.tensor_tensor(
                        act8[:, jj // 2, jj % 2], sg[:, :NF8], psu[:, :NF8],
                        mybir.AluOpType.mult,
                    )

            # ---- phase 1 mm2: y^T = W2^T act ----
            for d in range(DO):
                ws = w2pool.tile([P, FH, P], bf16, tag="w2t")
                nc.sync.dma_start(ws[:], w2[:, d])
                if NF8:
                    w28 = w2pool.tile([P, FH8, 2, P], fp8, tag="w28", bufs=2)
                    nc.scalar.dma_start(w28[:], w2f8[:, d])
                for ci, (t0, tw) in enumerate(tcs):
                    psy = psypool.tile([P, TC], f32, tag="psy")
                    for k2 in range(FH):
                        nc.tensor.matmul(
                            psy[:, :tw], ws[:, k2], act[:, k2, t0:t0 + tw],
                            start=(k2 == 0), stop=(k2 == FH - 1),
                        )
                    yo = ypool.tile([P, TC], f32, tag="yo")
                    nc.scalar.copy(yo[:, :tw], psy[:, :tw])
                    nc.sync.dma_start(yT[:, d, t0:t0 + tw], yo[:, :tw])
                if NF8:
                    psy = psypool.tile([P, TC], f32, tag="psy")
                    for kk in range(FH8):
                        nc.tensor.matmul(
                            psy[:, :NF8], w28[:, kk], act8[:, kk],
                            start=(kk == 0), stop=(kk == FH8 - 1),
                            perf_mode=DR,
                        )
                    yo = ypool.tile([P, TC], f32, tag="yo")
                    nc.scalar.copy(yo[:, :NF8], psy[:, :NF8])
                    nc.sync.dma_start(y8T[:, d], yo[:, :NF8])

            # ---- phase 2: leftover-token units (128 tok x 512 d_ff) ----
            # Its inputs ride the vector DMA queue, issued here so they land
            # behind the fp8 weight stream but well before phase-2 compute.
            if UPC:
                NPRE = min(UPC, 3)
                x2 = xpool.tile([P, KO1, UPC * P], bf16, tag="x2")
                nc.scalar.dma_start(x2[:], x2T[:])
                w1u_t, w2u_t = [], []
                for u in range(NPRE):
                    w1u = w1pool.tile([P, NJ2, 2, KO1, P], bf16,
                                      tag=f"w1s{u}", bufs=1)
                    nc.scalar.dma_start(w1u[:], w1s[:, u])
                    w2u = w2pool.tile([P, FH2, DO, P], bf16,
                                      tag=f"w2s{u}", bufs=1)
                    nc.scalar.dma_start(w2u[:], w2s[:, u])
                    w1u_t.append(w1u)
                    w2u_t.append(w2u)
                for u in range(UPC):
                    if u < len(w1u_t):
                        w1u, w2u = w1u_t[u], w2u_t[u]
                    else:
                        w1u = w1pool.tile([P, NJ2, 2, KO1, P], bf16,
                                          tag="w1sr", bufs=2)
                        nc.scalar.dma_start(w1u[:], w1s[:, u])
                        w2u = w2pool.tile([P, FH2, DO, P], bf16,
                                          tag="w2sr", bufs=2)
                        nc.scalar.dma_start(w2u[:], w2s[:, u])
                    act2 = actpool.tile([P, FH2, P], bf16, tag="act2", bufs=2)
                    xu = x2[:, :, u * P:(u + 1) * P]
                    for jj in range(NJ2):
                        psg = pspool.tile([P, TC], f32, tag="ps")
                        psu = pspool.tile([P, TC], f32, tag="ps")
                        for k in range(KO1):
                            nc.tensor.matmul(
                                psg[:, :P], w1u[:, jj, 0, k], xu[:, k],
                                start=(k == 0), stop=(k == KO1 - 1),
                            )
                        for k in range(KO1):
                            nc.tensor.matmul(
                                psu[:, :P], w1u[:, jj, 1, k], xu[:, k],
                                start=(k == 0), stop=(k == KO1 - 1),
                            )
                        sg = sgpool.tile([P, TC], f32, tag="sg")
                        nc.scalar.activation(
                            sg[:, :P], psg[:, :P],
                            mybir.ActivationFunctionType.Silu,
                        )
                        nc.vector.tensor_tensor(
                            act2[:, jj], sg[:, :P], psu[:, :P],
                            mybir.AluOpType.mult,
                        )
                    for d in range(DO):
                        psy = psypool.tile([P, TC], f32, tag="psy")
                        for k2 in range(FH2):
                            nc.tensor.matmul(
                                psy[:, :P], w2u[:, k2, d], act2[:, k2],
                                start=(k2 == 0), stop=(k2 == FH2 - 1),
                            )
                        yo = ypool.tile([P, TC], f32, tag="yo")
                        nc.scalar.copy(yo[:, :P], psy[:, :P])
                        nc.sync.dma_start(y2T[:, u, d], yo[:, :P])
    return nc


def _plan(counts, nf8):
    """Pick C1 and the phase-2 unit count minimizing per-core token-equivs.
    C1 counts both the nf8 fp8 slots and the bf16 slots of phase 1."""
    cmax = max(counts)
    lo = max(nf8 + P, min(P, cmax))
    best = None
    for C1 in range(lo, max(cmax, lo) + 1):
        blocks = sum((max(c - C1, 0) + P - 1) // P for c in counts)
        units = blocks * NSL
        upc = (units + E - 1) // E
        cost = C1 + (P * FSL // F) * upc  # C1 + 16 * units-per-core
        if best is None or cost < best[0] or (cost == best[0] and C1 > best[1]):
            best = (cost, C1, upc)
    _, C1, upc = best
    return C1, upc


def _pack_w1_phase1(w1d):
    # w1d [D, 2F] -> [P, NJ, 2, KO1, P]
    g = w1d[:, :F].reshape(KO1, P, NJ, P)       # [k, p, jj, c]
    u = w1d[:, F:].reshape(KO1, P, NJ, P)
    out = np.stack([g.transpose(1, 2, 0, 3), u.transpose(1, 2, 0, 3)], axis=2)
    return np.ascontiguousarray(out)            # [p, jj, gu, k, c]


def _pack_w2_phase1(w2d):
    # w2d [F, D] -> [P, DO, FH, P]
    return np.ascontiguousarray(w2d.reshape(FH, P, DO, P).transpose(1, 2, 0, 3))


def _pack_xT(xe, C):
    # xe [C, D] -> [P, KO1, C]
    return np.ascontiguousarray(xe.T.reshape(KO1, P, C).transpose(1, 0, 2))


def kernel(x, gating_output, w1_q, w2_q, w1_scale, w2_scale):
    global LAST_RESULTS
    from concourse.bass_utils import run_bass_kernel_spmd

    x = np.asarray(x, np.float32)
    w1_q = np.asarray(w1_q)
    w2_q = np.asarray(w2_q)
    w1_scale = np.asarray(w1_scale, np.float32)
    w2_scale = np.asarray(w2_scale, np.float32)

    topk_w, topk_ids = _route(gating_output)

    ph1_t, ph1_c, f8_t, f8_c, sp_t, sp_c = [], [], [], [], [], []
    counts = []
    for e in range(E):
        mask = topk_ids == e
        tok = np.nonzero(mask.any(axis=1))[0]
        cf = np.where(mask, topk_w, 0.0).sum(axis=1)[tok].astype(np.float32)
        order = np.argsort(cf, kind="stable")   # ascending coef
        tok, cf = tok[order], cf[order]
        counts.append(len(tok))
        nf8 = min(F8, len(tok))
        f8_t.append(tok[:nf8])
        f8_c.append(cf[:nf8])
        ph1_t.append(tok[nf8:])   # provisional; split below once C1 known
        ph1_c.append(cf[nf8:])

    C1, UPC = _plan(counts, F8)
    NB1 = C1 - F8
    for e in range(E):
        rest_t, rest_c = ph1_t[e], ph1_c[e]
        take = min(NB1, len(rest_t))
        cut = len(rest_t) - take
        sp_t.append(rest_t[:cut])
        sp_c.append(rest_c[:cut])
        ph1_t[e], ph1_c[e] = rest_t[cut:], rest_c[cut:]

    key = (C1, UPC, F8)
    if key not in _BUILD_CACHE:
        nc = _build(C1, UPC, F8)
        nc.finalize()
        _BUILD_CACHE[key] = nc
    nc = _BUILD_CACHE[key]

    # host-side dequant (once per expert)
    w1ds = [_dequant_bf16(w1_q[e], w1_scale[e]) for e in range(E)]  # [D, 2F]
    w2ds = [_dequant_bf16(w2_q[e], w2_scale[e]) for e in range(E)]  # [F, D]

    def to_e4m3(a):
        return np.asarray(np.clip(np.asarray(a, np.float32), -240.0, 240.0),
                          ml_dtypes.float8_e4m3fn)

    # phase-2 unit list: (expert, block_start_in_spill_list, slice)
    units = []
    for e in range(E):
        for b0 in range(0, len(sp_t[e]), P):
            for s in range(NSL):
                units.append((e, b0, s))
    per_core_units = [[] for _ in range(E)]
    for i, unit in enumerate(units):
        per_core_units[i % E].append(unit)

    in_maps = []
    for c in range(E):
        tok = ph1_t[c]
        xe = np.zeros((NB1, D), np.float32)
        xe[: len(tok)] = x[tok]
        w1f = np.asarray(w1ds[c], np.float32)
        w2f = np.asarray(w2ds[c], np.float32)
        x8e = np.zeros((F8, D), np.float32)
        x8e[: len(f8_t[c])] = x[f8_t[c]]
        g8 = to_e4m3(w1f[:, :F]).reshape(KO2, 2, P, NJ, P)
        u8 = to_e4m3(w1f[:, F:]).reshape(KO2, 2, P, NJ, P)
        im = {
            "xT": _pack_xT(xe.astype(ml_dtypes.bfloat16), NB1),
            "w1": _pack_w1_phase1(w1ds[c]),
            "w2": _pack_w2_phase1(w2ds[c]),
            "x8T": np.ascontiguousarray(
                to_e4m3(x8e).T.reshape(KO2, 2, P, F8).transpose(2, 0, 1, 3)),
            "w1f8": np.ascontiguousarray(np.stack(
                [g8.transpose(2, 3, 0, 1, 4), u8.transpose(2, 3, 0, 1, 4)],
                axis=2)),
            "w2f8": np.ascontiguousarray(
                to_e4m3(w2f).reshape(FH8, 2, P, DO, P).transpose(2, 3, 0, 1, 4)),
        }
        if UPC:
            x2 = np.zeros((UPC * P, D), np.float32)
            w1s = np.zeros((P, UPC, NJ2, 2, KO1, P), ml_dtypes.bfloat16)
            w2s = np.zeros((P, UPC, FH2, DO, P), ml_dtypes.bfloat16)
            for j, (e, b0, s) in enumerate(per_core_units[c]):
                btok = sp_t[e][b0:b0 + P]
                x2[j * P: j * P + len(btok)] = x[btok]
                w1d = w1ds[e]
                gs = w1d[:, s * FSL:(s + 1) * FSL].reshape(KO1, P, NJ2, P)
                us = w1d[:, F + s * FSL: F + (s + 1) * FSL].reshape(KO1, P, NJ2, P)
                w1s[:, j] = np.stack(
                    [gs.transpose(1, 2, 0, 3), us.transpose(1, 2, 0, 3)], axis=2
                )
                w2sl = w2ds[e][s * FSL:(s + 1) * FSL]  # [FSL, D]
                w2s[:, j] = w2sl.reshape(FH2, P, DO, P).transpose(1, 0, 2, 3)
            im["x2T"] = _pack_xT(x2.astype(ml_dtypes.bfloat16), UPC * P)
            im["w1s"] = w1s
            im["w2s"] = w2s
        in_maps.append(im)

    LAST_RESULTS = run_bass_kernel_spmd(nc, in_maps, core_ids=list(range(E)))

    out = np.zeros((T, D), np.float32)
    for c in range(E):
        yTe = LAST_RESULTS.results[c]["yT"]          # [P, DO, NB1] f32
        y = yTe.transpose(1, 0, 2).reshape(D, NB1).T
        tok = ph1_t[c]
        out[tok] += ph1_c[c][:, None] * y[: len(tok)]
        y8e = LAST_RESULTS.results[c]["y8T"]         # [P, DO, F8] f32
        y8 = y8e.transpose(1, 0, 2).reshape(D, F8).T
        out[f8_t[c]] += f8_c[c][:, None] * y8[: len(f8_t[c])]
        if UPC:
            y2Te = LAST_RESULTS.results[c]["y2T"]    # [P, UPC, DO, P] f32
            for j, (e, b0, s) in enumerate(per_core_units[c]):
                btok = sp_t[e][b0:b0 + P]
                y2 = y2Te[:, j].transpose(1, 0, 2).reshape(D, P).T  # [P, D]
                out[btok] += sp_c[e][b0:b0 + len(btok), None] * y2[: len(btok)]
    return out
